# revision 35
# baseline (speedup 1.0000x reference)
"""Trainium2 Bass kernel for nn_Classifiers_18133351923788.

Two-branch LSTM classifier, data-parallel over 8 NeuronCores (1024 batch rows
per core). On-chip layout is feature-major (features on partitions, batch on
the free dim); all matmuls are bf16 with fp32 PSUM accumulation.

Per-core pipeline:
  1. gpsimd DMA casts the fp32 input shard to a bf16 DRAM bounce buffer
     (one DMA per branch; ac first since it unblocks dense0 soonest).
  2. HWDGE xbar-transpose DMAs produce x^T chunk tiles [128 feat, 512 batch]
     in SBUF, streamed through a small ring. All weights ship in two packed
     tensors (single DMA each) so they don't clog the HWDGE queue ahead of
     the transposes; the xbar-mode-transition rule serializes every
     transpose behind in-flight DMA copies, so copies are batched.
  3. dense0+BN+relu: 4 K-chunk matmul accumulation + one ACT op per tile
     (relu with per-partition scale/bias; BN folded on the host).
  4. Three stacked LSTM layers per branch; W and U parts accumulate in the
     same PSUM group per step; sigmoid gates are packed ACT ops (i,f share
     one op), relu(g) runs on DVE, f*c on gpsimd; lstm2 biases ride the W
     matmul via a ones-row in h1. Cell state lives on partitions 64:128 so
     DVE tensor_tensor sees equal base partitions. relu(c) == c because
     g = relu(.) >= 0 and c_0 = i*g >= 0. In the final waves the offloads
     revert to the lowest-latency engines (chain-bound, not
     throughput-bound).
  5. Heads: dense+relu, dense, PE-transpose of logits to batch-major,
     softmax (exp/sum/recip/mul), fp32 DMA out.
The schedule runs dense0-ac, then joint waves {ac tower step, dense0-cc
t-step, cc tower step one wave behind} so the tensor engine stays busy while
each branch waits on its pointwise chain.

Cost-model timeline (single core): ~162 us, PE busy ~110 us (bottleneck),
ACT ~93 us, DVE ~72 us, Pool ~41 us, DMA ~58 us. Down from 228 us for the
first correct version.
"""
import sys

sys.path.insert(0, "/opt/trn_rl_repo")

import numpy as np
import ml_dtypes

import concourse.bass as bass
import concourse.mybir as mybir
import concourse.tile as tile
from concourse.bass_utils import run_bass_kernel_spmd

BF16 = ml_dtypes.bfloat16
DT_BF = mybir.dt.bfloat16
DT_F32 = mybir.dt.float32
AF = mybir.ActivationFunctionType

N_CORES = 8
B_LOC = 1024          # batch rows per core
T_AC, T_CC = 3, 5
F_IN = 512
D0 = 300              # dense0 output features
BN_EPS = 1e-3
NC2 = 512             # matmul free-dim chunk
NCHUNK = B_LOC // NC2  # batch chunks per core

# dense0 output M-chunks (also lstm1 W K-chunks)
D0_CHUNKS = [(0, 128), (128, 128), (256, 44)]

# vecs tensor column layout (per branch), fp32 [128, 16]
VC_SCALE0 = 0   # cols 0..2
VC_SBIAS0 = 3   # cols 3..5
VC_B1 = 6       # cols 6..7
VC_B2 = 8       # cols 8..11
VC_B3 = 12      # cols 12..13
VC_BD1 = 14
VC_BD2 = 15

# packed bf16 weight layout: name -> (col offset, width, partitions)
# (w0 ships separately: it is the only weight needed before dense0 starts)
_WOFF = {}
_off = 0
for _nm, _w, _p in [
        (f"{n}_{b}", w, p) for b in ("ac", "cc")
        for n, w, p in (("w1", 768, 128), ("u1", 256, 64), ("w2", 512, 65),
                        ("u2", 512, 128), ("w3", 256, 128), ("u3", 256, 64),
                        ("d1", 100, 64), ("d2", 16, 100))]:
    _WOFF[_nm] = (_off, _w, _p)
    _off += _w
WPACK_COLS = _off
W0_COLS = 1200


def _split_multi_waits(nc, max_waits=1):
    """This walrus build accepts at most one semaphore wait per instruction.
    Hoist extra waits onto no-op carriers inserted just before the owner."""
    for f in nc.m.functions:
        for bb in f.blocks:
            insts = list(bb.instructions)
            out = []
            changed = False
            for inst in insts:
                si = inst.sync_info
                if si is not None and si.on_wait and len(si.on_wait) > max_waits:
                    waits = list(si.on_wait)
                    ups = list(si.on_update) if si.on_update else []
                    head, tail = waits[:-max_waits], waits[-max_waits:]
                    for i in range(0, len(head), max_waits):
                        nop = mybir.InstNoOp(
                            name=f"I-waitsplit-{nc.next_id()}",
                            engine=inst.engine,
                            sync_info=mybir.SyncInfo(
                                on_wait=head[i:i + max_waits], on_update=[]),
                            bass_nofuse=True,
                        )
                        out.append(nop)
                    inst.sync_info = mybir.SyncInfo(on_wait=tail, on_update=ups)
                    changed = True
                out.append(inst)
            if changed:
                bb.instructions = out


def _build_core_program():
    nc = bass.Bass()

    # ---------------- external tensors ----------------
    x_ac = nc.declare_dram_parameter("x_ac", [B_LOC, T_AC, F_IN], DT_F32, isOutput=False)
    x_cc = nc.declare_dram_parameter("x_cc", [B_LOC, T_CC, F_IN], DT_F32, isOutput=False)

    # bf16 weights packed into two tensors (w0 alone, rest together);
    # fp32 vectors (+ identity) packed into one [128, 48] tensor
    w0pack = nc.declare_dram_parameter("w0pack", [128, W0_COLS], DT_BF,
                                       isOutput=False)
    wpack = nc.declare_dram_parameter("wpack", [128, WPACK_COLS], DT_BF,
                                      isOutput=False)
    vpack = nc.declare_dram_parameter("vpack", [128, 48], DT_F32,
                                      isOutput=False)
    ones = nc.declare_dram_parameter("ones", [1, T_CC * B_LOC], DT_BF,
                                     isOutput=False)

    out_ac = nc.declare_dram_parameter("out_ac", [B_LOC, 11], DT_F32, isOutput=True)
    out_cc = nc.declare_dram_parameter("out_cc", [B_LOC, 10], DT_F32, isOutput=True)

    # bf16 bounce copies of the inputs (for the 2-byte xbar transpose);
    # one tensor per batch half so transpose deps don't serialize on the
    # whole cast
    xbf = {br: [nc.dram_tensor(f"xbf_{br}", [B_LOC, T, F_IN], DT_BF)]
           for br, T in (("ac", T_AC), ("cc", T_CC))}

    BRANCH = {
        "ac": dict(T=T_AC, x=x_ac, out=out_ac, C=11, coloff=0),
        "cc": dict(T=T_CC, x=x_cc, out=out_cc, C=10, coloff=T_AC * B_LOC),
    }
    TOT_COLS = (T_AC + T_CC) * B_LOC  # 8192

    with tile.TileContext(nc) as tc:
        with (
            tc.tile_pool(name="wpool", bufs=1) as wpool,
            tc.tile_pool(name="xt", bufs=4) as xt_pool,
            tc.tile_pool(name="ypool", bufs=1) as y_pool,
            tc.tile_pool(name="hseq", bufs=1) as h_pool,
            tc.tile_pool(name="cpool", bufs=2) as c_pool,
            tc.tile_pool(name="gates", bufs=2) as g_pool,
            tc.tile_pool(name="dve", bufs=2) as d_pool,
            tc.tile_pool(name="misc", bufs=3) as m_pool,
            tc.tile_pool(name="ps1", bufs=2, space="PSUM") as ps1,
            tc.tile_pool(name="psz", bufs=3, space="PSUM") as psz,
        ):
            # ------------- weight loads -------------
            # phase-0 loads: only what dense0-ac needs (w0 + vectors)
            w0p_sb = wpool.tile([128, W0_COLS], DT_BF, tag="w0pack")
            nc.scalar.dma_start(w0p_sb[:], w0pack[:])
            vp_sb = wpool.tile([128, 48], DT_F32, tag="vpack")
            nc.scalar.dma_start(vp_sb[:], vpack[:])
            w0_sb = w0p_sb[:, :].rearrange("p (c m) -> p c m", m=D0)
            wp_sb = wpool.tile([128, WPACK_COLS], DT_BF, tag="wpack")

            W = {}
            for nm, (off, width, parts) in _WOFF.items():
                ap = wp_sb[:parts, off:off + width]
                if nm.startswith("w1"):
                    ap = ap.rearrange("p (c m) -> p c m", m=256)
                W[nm] = ap
            W["vecs_ac"] = vp_sb[:, 0:16]
            W["vecs_cc"] = vp_sb[:, 16:32]
            eye_sb = vp_sb[:16, 32:48]

            # ------------- dense0 + BN + relu (with streamed transposes) ----
            # y tiles: feature-major [chunk, 8192] bf16 (ac cols then cc cols)
            Y = []
            for mc, (m0, msz) in enumerate(D0_CHUNKS):
                Y.append(y_pool.tile([msz, TOT_COLS], DT_BF, tag=f"y{mc}", name=f"y{mc}"))

            last_transpose = [None]

            def emit_dense0_pair(br, p, paired=True):
                """One t-step of dense0 (both batch halves): transposes +
                matmuls + fused relu/BN ACT per M-chunk. paired=True uses a
                2-bank psz tile with one ACT; paired=False uses 1-bank ps1
                tiles (2 ACTs) to keep psz free for concurrent tower steps."""
                info = BRANCH[br]
                vecs = W[f"vecs_{br}"]
                xts = []
                for j in range(2):
                    n = 2 * p + j
                    t, nb = n // NCHUNK, n % NCHUNK
                    src = xbf[br][0]  # [B, T, 512] bf16
                    bs = slice(nb * NC2, (nb + 1) * NC2)
                    xtc = xt_pool.tile([128, 4, NC2], DT_BF, tag="xtc",
                                       name="xtc")
                    for fc in range(4):
                        ti = nc.sync.dma_start(
                            out=xtc[:, fc, :],
                            in_=src[bs, t, fc * 128:(fc + 1) * 128],
                            transpose=True,
                        )
                        last_transpose[0] = ti
                    xts.append(xtc)
                col0 = info["coloff"] + 2 * p * NC2
                for mc, (m0, msz) in enumerate(D0_CHUNKS):
                    if paired:
                        ps = psz.tile([128, 2, NC2], DT_F32, tag="psz",
                                      name="d0ps")
                        for j in range(2):
                            for kc in range(4):
                                nc.tensor.matmul(
                                    ps[:msz, j, :],
                                    w0_sb[:, kc, m0:m0 + msz],
                                    xts[j][:, kc, :],
                                    start=(kc == 0), stop=(kc == 3),
                                )
                        nc.scalar.activation(
                            Y[mc][:, col0:col0 + 2 * NC2], ps[:msz, :, :],
                            AF.Relu,
                            bias=vecs[:msz, VC_SBIAS0 + mc:VC_SBIAS0 + mc + 1],
                            scale=vecs[:msz, VC_SCALE0 + mc:VC_SCALE0 + mc + 1],
                        )
                    else:
                        for j in range(2):
                            ps = ps1.tile([128, NC2], DT_F32, tag="ps1",
                                          name="d0ps1")
                            for kc in range(4):
                                nc.tensor.matmul(
                                    ps[:msz, :],
                                    w0_sb[:, kc, m0:m0 + msz],
                                    xts[j][:, kc, :],
                                    start=(kc == 0), stop=(kc == 3),
                                )
                            c0 = col0 + j * NC2
                            nc.scalar.activation(
                                Y[mc][:, c0:c0 + NC2], ps[:msz, :],
                                AF.Relu,
                                bias=vecs[:msz, VC_SBIAS0 + mc:VC_SBIAS0 + mc + 1],
                                scale=vecs[:msz, VC_SCALE0 + mc:VC_SCALE0 + mc + 1],
                            )

            # ------------- LSTM towers (wavefront interleaved) -------------
            H = {}
            Cprev = {}
            for br in ("ac", "cc"):
                T = BRANCH[br]["T"]
                # h1 carries a ones-row at partition 64: the lstm2 W matmul
                # contracts over 65 rows, adding the (host-folded) lstm2 bias.
                H[f"h1_{br}"] = h_pool.tile([65, T * B_LOC], DT_BF, tag=f"h1_{br}", name=f"h1_{br}")
                nc.scalar.dma_start(H[f"h1_{br}"][64:65, :], ones[:, :T * B_LOC])
                H[f"h2_{br}"] = h_pool.tile([128, T * B_LOC], DT_BF, tag=f"h2_{br}", name=f"h2_{br}")
                H[f"h3_{br}"] = h_pool.tile([64, B_LOC], DT_BF, tag=f"h3_{br}", name=f"h3_{br}")
                for l in (1, 2, 3):
                    Cprev[f"{br}{l}"] = None

            LAYER = {
                1: dict(units=64, b_col=VC_B1),
                2: dict(units=128, b_col=VC_B2),
                3: dict(units=64, b_col=VC_B3),
            }

            AL = mybir.AluOpType

            def emit_lstm_64(br, l, t, late=False):
                """units-64 layer (lstm1/lstm3), full-batch FD=1024 pointwise.

                Layout trick: c, o, and the ig/fc temporaries live on
                partitions 64:128 so that every DVE tensor_tensor sees both
                SBUF operands at the same base partition (i,g at base 0;
                f,c,o at base 64). relu g runs on DVE (tensor_scalar
                add-bias + max-0); sigmoid i,f share one ACT op."""
                info = BRANCH[br]
                T = info["T"]
                b_col = LAYER[l]["b_col"]
                vecs = W[f"vecs_{br}"]
                w_sb = W[f"w{l}_{br}"]
                u_sb = W[f"u{l}_{br}"]

                h_out = H[f"h1_{br}"] if l == 1 else H[f"h3_{br}"]
                h_prev_seq = H[f"h{l - 1}_{br}"] if l > 1 else None
                h_rec = H.get(f"h3prev_{br}") if l == 3 else h_out

                c_new = c_pool.tile([128, B_LOC], DT_BF, tag=f"c{l}_{br}",
                                    name=f"c{l}_{br}")
                c_old = Cprev[f"{br}{l}"]
                h3_new = None
                if l == 3 and t < T - 1:
                    h3_new = c_pool.tile([64, B_LOC], DT_BF, tag=f"h3tmp_{br}",
                                         name=f"h3tmp_{br}")

                # z formation: zif holds gates [i|f], zgo holds [g|o]
                zs = []
                for mc in range(2):
                    ps = psz.tile([128, NCHUNK, NC2], DT_F32, tag="psz",
                                  name=f"z{l}_{br}")
                    msl = slice(mc * 128, (mc + 1) * 128)
                    for n in range(NCHUNK):
                        cur = t * B_LOC + n * NC2
                        mms = []
                        if l == 1:
                            ycols = slice(info["coloff"] + cur,
                                          info["coloff"] + cur + NC2)
                            for kc, (k0, ksz) in enumerate(D0_CHUNKS):
                                mms.append((w_sb[:ksz, kc, msl], Y[kc][:, ycols]))
                        else:
                            mms.append((w_sb[:, msl],
                                        h_prev_seq[:, cur:cur + NC2]))
                        if t > 0:
                            rs = (slice(n * NC2, (n + 1) * NC2) if l == 3 else
                                  slice(cur - B_LOC, cur - B_LOC + NC2))
                            mms.append((u_sb[:64, msl], h_rec[:64, rs]))
                        for idx, (a, b) in enumerate(mms):
                            nc.tensor.matmul(ps[:, n, :], a, b,
                                             start=(idx == 0),
                                             stop=(idx == len(mms) - 1))
                    zs.append(ps)

                # gates + cell/hidden update; c and temporaries live on
                # partitions 64:128. Early waves process the full batch per
                # op (throughput); late waves go per-chunk so chunk 0's h
                # unblocks the next step's recurrent matmul sooner (latency).
                if_sb = g_pool.tile([128, B_LOC], DT_BF, tag="g_if",
                                    name="g_if")
                g_sb = g_pool.tile([64, B_LOC], DT_BF, tag="g_g", name="g_g")
                o_sb = g_pool.tile([128, B_LOC], DT_BF, tag="g_o", name="g_o")
                ig = fc_ = None
                if t > 0:
                    ig = d_pool.tile([128, B_LOC], DT_BF, tag="t_ig", name="t_ig")
                    fc_ = d_pool.tile([128, B_LOC], DT_BF, tag="t_fc", name="t_fc")
                chunks = [slice(n * NC2, (n + 1) * NC2) for n in range(NCHUNK)] \
                    if late else [slice(0, B_LOC)]
                for n, cs in enumerate(chunks):
                    zsl = (slice(n, n + 1) if late else slice(0, NCHUNK))
                    nc.scalar.activation(if_sb[:, cs], zs[0][:, zsl, :],
                                         AF.Sigmoid,
                                         bias=vecs[:, b_col:b_col + 1])
                    if late:
                        nc.scalar.activation(
                            g_sb[:, cs], zs[1][:64, zsl, :], AF.Relu,
                            bias=vecs[:64, b_col + 1:b_col + 2])
                    else:
                        nc.vector.tensor_scalar(
                            g_sb[:, cs], zs[1][:64, zsl, :],
                            vecs[:64, b_col + 1:b_col + 2],
                            0.0, op0=AL.add, op1=AL.max)
                    nc.scalar.activation(o_sb[64:128, cs], zs[1][64:128, zsl, :],
                                         AF.Sigmoid,
                                         bias=vecs[64:128, b_col + 1:b_col + 2])
                    if t == 0:
                        nc.vector.tensor_mul(c_new[64:128, cs], if_sb[:64, cs],
                                             g_sb[:, cs])
                    else:
                        nc.vector.tensor_mul(ig[64:128, cs], if_sb[:64, cs],
                                             g_sb[:, cs])
                        eng = nc.vector if late else nc.gpsimd
                        eng.tensor_mul(fc_[64:128, cs], if_sb[64:128, cs],
                                       c_old[64:128, cs])
                        nc.vector.tensor_add(c_new[64:128, cs], ig[64:128, cs],
                                             fc_[64:128, cs])
                    if l == 3 and t < T - 1:
                        nc.vector.tensor_mul(h3_new[:, cs], o_sb[64:128, cs],
                                             c_new[64:128, cs])
                    elif l == 3:
                        nc.vector.tensor_mul(h_out[:, cs], o_sb[64:128, cs],
                                             c_new[64:128, cs])
                    else:
                        hc = slice(t * B_LOC + cs.start, t * B_LOC + cs.stop)
                        nc.vector.tensor_mul(h_out[:64, hc],
                                             o_sb[64:128, cs], c_new[64:128, cs])

                Cprev[f"{br}{l}"] = c_new
                if l == 3 and t < T - 1:
                    H[f"h3prev_{br}"] = h3_new

            def emit_lstm_128(br, t, late=False):
                """units-128 layer (lstm2), per-chunk pointwise; i,f share a
                2-bank PSUM tile so one sigmoid ACT covers both."""
                info = BRANCH[br]
                b_col = LAYER[2]["b_col"]
                vecs = W[f"vecs_{br}"]
                w_sb = W[f"w2_{br}"]
                u_sb = W[f"u2_{br}"]
                h_out = H[f"h2_{br}"]
                h_prev_seq = H[f"h1_{br}"]

                c_new = c_pool.tile([128, B_LOC], DT_BF, tag=f"c2_{br}",
                                    name=f"c2_{br}")
                c_old = Cprev[f"{br}2"]

                for n in range(NCHUNK):
                    cs = slice(n * NC2, (n + 1) * NC2)
                    cur = t * B_LOC + n * NC2
                    curs = slice(cur, cur + NC2)
                    prevs = slice(cur - B_LOC, cur - B_LOC + NC2)

                    zs = []
                    for pair in range(2):  # 0: (i,f)  1: (g,o)
                        ps = psz.tile([128, 2, NC2], DT_F32, tag="psz",
                                      name=f"z2_{br}")
                        for gi in range(2):
                            msl = slice((pair * 2 + gi) * 128,
                                        (pair * 2 + gi + 1) * 128)
                            mms = [(w_sb[:, msl], h_prev_seq[:, curs])]
                            if t > 0:
                                mms.append((u_sb[:, msl], h_out[:, prevs]))
                            for idx, (a, b) in enumerate(mms):
                                nc.tensor.matmul(ps[:, gi, :], a, b,
                                                 start=(idx == 0),
                                                 stop=(idx == len(mms) - 1))
                        zs.append(ps)

                    # lstm2 biases ride the W matmul via h1's ones-row, so a
                    # single sigmoid covers i and f and no bias operands are
                    # needed.
                    if2 = g_pool.tile([128, 2, NC2], DT_BF, tag="g_if2",
                                      name="g_if2")
                    nc.scalar.activation(if2[:], zs[0][:], AF.Sigmoid)
                    g2 = g_pool.tile([128, NC2], DT_BF, tag="g_g2", name="g_g2")
                    if late:
                        nc.scalar.activation(g2[:], zs[1][:, 0, :], AF.Relu)
                    else:
                        nc.vector.tensor_scalar(
                            g2[:], zs[1][:, 0, :], 0.0, None, op0=AL.max)
                    o2 = g_pool.tile([128, NC2], DT_BF, tag="g_o2", name="g_o2")
                    nc.scalar.activation(o2[:], zs[1][:, 1, :], AF.Sigmoid)

                    if t == 0:
                        nc.vector.tensor_mul(c_new[:, cs], if2[:, 0, :], g2[:])
                    else:
                        ig = d_pool.tile([128, NC2], DT_BF, tag="t_ig2",
                                         name="t_ig2")
                        fc_ = d_pool.tile([128, NC2], DT_BF, tag="t_fc2",
                                          name="t_fc2")
                        nc.vector.tensor_mul(ig[:], if2[:, 0, :], g2[:])
                        eng = nc.vector if late else nc.gpsimd
                        eng.tensor_mul(fc_[:], if2[:, 1, :], c_old[:, cs])
                        nc.vector.tensor_add(c_new[:, cs], ig[:], fc_[:])
                    nc.vector.tensor_mul(h_out[:, curs], o2[:], c_new[:, cs])

                Cprev[f"{br}2"] = c_new

            def emit_lstm_step(br, l, t, late=False):
                if l == 2:
                    emit_lstm_128(br, t, late)
                else:
                    emit_lstm_64(br, l, t, late)

            def emit_head(br):
                info = BRANCH[br]
                C = info["C"]
                vecs = W[f"vecs_{br}"]
                d1_sb, d2_sb = W[f"d1_{br}"], W[f"d2_{br}"]
                h3 = H[f"h3_{br}"]
                for n in range(NCHUNK):
                    cs = slice(n * NC2, (n + 1) * NC2)
                    psu = ps1.tile([128, NC2], DT_F32, tag="ps1")
                    nc.tensor.matmul(psu[:100, :], d1_sb[:64, :], h3[:, cs])
                    u_sb = m_pool.tile([100, NC2], DT_BF, tag="u_sb")
                    nc.scalar.activation(u_sb[:], psu[:100, :], AF.Relu,
                                         bias=vecs[:100, VC_BD1:VC_BD1 + 1])
                    psl = ps1.tile([128, NC2], DT_F32, tag="ps1")
                    nc.tensor.matmul(psl[:C, :], d2_sb[:, :C], u_sb[:])
                    logit = m_pool.tile([C, NC2], DT_F32, tag="logit")
                    nc.scalar.activation(logit[:], psl[:C, :], AF.Identity,
                                         bias=vecs[:C, VC_BD2:VC_BD2 + 1])
                    # transpose to batch-major [128, 4, C]
                    pst = ps1.tile([128, 4, C], DT_F32, tag="ps1")
                    for j in range(4):
                        nc.tensor.transpose(
                            pst[:, j, :], logit[:, j * 128:(j + 1) * 128],
                            eye_sb[:C, :C])
                    exp_sb = m_pool.tile([128, 4, C], DT_F32, tag="exp")
                    nc.scalar.activation(exp_sb[:], pst[:], AF.Exp)
                    ssum = d_pool.tile([128, 4], DT_F32, tag="ssum")
                    nc.vector.reduce_sum(ssum[:], exp_sb[:], axis=mybir.AxisListType.X)
                    rsum = d_pool.tile([128, 4], DT_F32, tag="rsum")
                    nc.vector.reciprocal(rsum[:], ssum[:])
                    prob = m_pool.tile([128, 4, C], DT_F32, tag="prob")
                    for j in range(4):
                        nc.vector.tensor_scalar_mul(
                            prob[:, j, :], exp_sb[:, j, :], rsum[:, j:j + 1])
                    # out rows b = n*512 + j*128 + p
                    dst = info["out"].rearrange("(n j p) c -> n p j c",
                                                n=NCHUNK, j=4, p=128)[n]
                    nc.sync.dma_start(dst, prob[:])

            # Schedule: dense0-ac first; then joint waves — each wave emits
            # the ac tower step, a dense0-cc t-step (fills PE while pointwise
            # chains run), and the cc tower step one wave behind its dense0.
            # ac cast, then dense0-ac (its transposes), then the bulk weight
            # load + cc cast (copies batched to minimize xbar transitions)
            nc.gpsimd.dma_start(xbf["ac"][0][:], BRANCH["ac"]["x"][:])
            for p in range(T_AC):
                emit_dense0_pair("ac", p)
            # hard-defer the bulk weight load and cc cast behind the last ac
            # transpose: the scheduler otherwise hoists these dependency-free
            # copies first and the xbar-transition rule then serializes every
            # transpose behind ~19us of copy traffic
            ac_tr = last_transpose[0]
            wi = nc.scalar.dma_start(wp_sb[:], wpack[:])
            bass._add_dep_helper(wi.ins, ac_tr.ins, sync=True,
                                 reason="defer wpack past ac transposes")
            ci = nc.gpsimd.dma_start(xbf["cc"][0][:], BRANCH["cc"]["x"][:])
            bass._add_dep_helper(ci.ins, ac_tr.ins, sync=True,
                                 reason="defer cc cast past ac transposes")

            def emit_wave(br, w, late=False):
                T = BRANCH[br]["T"]
                for l in (1, 2, 3):
                    t = w - (l - 1)
                    if 0 <= t < T:
                        emit_lstm_step(br, l, t, late)

            for w in range(2 + T_CC + 1):  # 0..7
                if w < 2 + T_AC:
                    emit_wave("ac", w)
                if w < T_CC:
                    emit_dense0_pair("cc", w, paired=False)
                if w >= 1:
                    emit_wave("cc", w - 1, late=(w - 1 >= 5))
                if w == 2 + T_AC:
                    emit_head("ac")
            emit_head("cc")

    _split_multi_waits(nc, max_waits=1)
    return nc


_NC = None


def _get_nc():
    global _NC
    if _NC is None:
        _NC = _build_core_program()
    return _NC


def _prep_params(params):
    """Host-side folding; returns {"wpack": ..., "vpack": ...}."""
    P = {k: {kk: np.asarray(vv, np.float32) for kk, vv in v.items()}
         for k, v in params.items()}
    wpack = np.zeros((128, WPACK_COLS), np.float32)
    vpack = np.zeros((128, 48), np.float32)

    def put(nm, arr):
        off, width, parts = _WOFF[nm]
        assert arr.shape == (parts, width), (nm, arr.shape)
        wpack[:parts, off:off + width] = arr

    W0 = P["dense0_ac"]["W"]
    w0pack = np.concatenate([W0[c * 128:(c + 1) * 128, :] for c in range(4)], 1)
    vpack[:16, 32:48] = np.eye(16, dtype=np.float32)

    for vi, br in enumerate(("ac", "cc")):
        bn = P[f"bn_{br}"]
        s = bn["gamma"] / np.sqrt(bn["var"] + BN_EPS)
        t_sh = bn["beta"] - bn["mean"] * s
        assert (s > 0).all()
        b0 = P["dense0_ac"]["b"]

        l1, l2, l3 = P[f"lstm1_{br}"], P[f"lstm2_{br}"], P[f"lstm3_{br}"]
        d1, d2 = P[f"d1_{br}"], P[f"d2_{br}"]

        w1 = np.zeros((384, 256), np.float32)
        w1[:300] = l1["W"]
        put(f"w1_{br}", np.concatenate(
            [w1[c * 128:(c + 1) * 128, :] for c in range(3)], 1))
        put(f"u1_{br}", l1["U"])
        put(f"w2_{br}", np.vstack([l2["W"], l2["b"][None, :]]))
        put(f"u2_{br}", l2["U"])
        put(f"w3_{br}", l3["W"])
        put(f"u3_{br}", l3["U"])
        put(f"d1_{br}", d1["W"])
        C = d2["W"].shape[1]
        d2w = np.zeros((100, 16), np.float32)
        d2w[:, :C] = d2["W"]
        put(f"d2_{br}", d2w)

        b1 = l1["b"] + t_sh @ l1["W"]
        vecs = np.zeros((128, 16), np.float32)
        for mc, (m0, msz) in enumerate(D0_CHUNKS):
            vecs[:msz, VC_SCALE0 + mc] = s[m0:m0 + msz]
            vecs[:msz, VC_SBIAS0 + mc] = (s * b0)[m0:m0 + msz]
        vecs[:, VC_B1] = b1[0:128]
        vecs[:, VC_B1 + 1] = b1[128:256]
        vecs[:, VC_B3] = l3["b"][0:128]
        vecs[:, VC_B3 + 1] = l3["b"][128:256]
        vecs[:100, VC_BD1] = d1["b"]
        vecs[:C, VC_BD2] = d2["b"]
        vpack[:, vi * 16:(vi + 1) * 16] = vecs

    return {"wpack": wpack.astype(BF16), "w0pack": w0pack.astype(BF16),
            "vpack": vpack,
            "ones": np.ones((1, T_CC * B_LOC), BF16)}


def kernel(input_ac, input_cc, params):
    input_ac = np.ascontiguousarray(np.asarray(input_ac, np.float32))
    input_cc = np.ascontiguousarray(np.asarray(input_cc, np.float32))
    wmaps = _prep_params(params)

    nc = _get_nc()
    in_maps = []
    for i in range(N_CORES):
        rows = slice(i * B_LOC, (i + 1) * B_LOC)
        m = {"x_ac": input_ac[rows], "x_cc": input_cc[rows]}
        m.update(wmaps)
        in_maps.append(m)

    res = run_bass_kernel_spmd(nc, in_maps, list(range(N_CORES)))
    global LAST_RESULT
    LAST_RESULT = res
    out_ac = np.concatenate([np.asarray(res.results[i]["out_ac"]) for i in range(N_CORES)], 0)
    out_cc = np.concatenate([np.asarray(res.results[i]["out_cc"]) for i in range(N_CORES)], 0)
    return (out_ac, out_cc)


LAST_RESULT = None
